# revision 2
# baseline (speedup 1.0000x reference)
"""Sliding-window causal self-attention (WINDOW=256) on 8 trn2 NeuronCores.

Sharding: 8 cores = 4 batch items x 2 sequence halves (1024 queries each).
Each core receives x pre-transposed with a 256-row key/value halo and
computes its output chunk fully independently; the host concatenates.

v2 layout (vs baseline): projections and attention are interleaved per
head-pair so ScalarE exp work hides under PE matmuls; Exp and Ln share
one activation-table set (no ACT table thrash); K/Q biases are fused
into the PSUM->SBUF copies as DVE tensor_scalar adds (no K=1 bias
matmuls); input DMAs are chunked and issued on two queues so the first
matmul starts ~2.5us in; output is written bf16 and widened on host.
"""

import numpy as np
import ml_dtypes

import concourse.bass as bass
import concourse.bacc as bacc
import concourse.bacc as bacc_mod
import concourse.mybir as mybir
from concourse.hw_specs import get_activation_tables
from concourse.tile import TileContext
from concourse.bass_utils import run_bass_kernel_spmd

F32 = mybir.dt.float32
BF16 = mybir.dt.bfloat16
AF = mybir.ActivationFunctionType
OP = mybir.AluOpType

N_HEAD = 12
WINDOW = 256
B, T, C = 4, 2048, 768
HD = C // N_HEAD              # 64
TQ = 1024                     # queries per core
HALO = 256
ROWS = TQ + HALO              # 1280 rows of k/v per core
NCT = C // 128                # 6 contraction tiles
NKT = ROWS // 128             # 10 key tiles
WIN = 384                     # q-window width per key tile
SCALE = 1.0 / float(np.sqrt(HD))

# q-window start per key tile (compile-time, same on every core)
QS = [min(max(128 * (kt - 2), 0), TQ - WIN) for kt in range(NKT)]

_TABLES_PATCHED = False


def _patch_act_tables():
    """Make Exp and Ln resolve only to natural_log_exp_and_others so the
    table-load inserter picks one set for both (no per-head reloads).
    Set ids/names are unchanged; only membership used for selection."""
    global _TABLES_PATCHED
    if _TABLES_PATCHED:
        return
    _TABLES_PATCHED = True

    def patched(arch):
        out = {}
        for name, fns in get_activation_tables(arch).items():
            fns = set(fns)
            if name != "natural_log_exp_and_others":
                fns.discard(AF.Exp)
                fns.discard(AF.Ln)
            out[name] = fns
        return out

    bacc_mod.get_activation_tables = patched


def _build_nc() -> bass.Bass:
    _patch_act_tables()
    nc = bacc.Bacc()

    # x_T in three column (token) chunks, host-sliced so DMAs are contiguous
    xt0_d = nc.dram_tensor("xt0", [C, 512], BF16, kind="ExternalInput")
    xt1_d = nc.dram_tensor("xt1", [C, 512], BF16, kind="ExternalInput")
    xt2_d = nc.dram_tensor("xt2", [C, 256], BF16, kind="ExternalInput")
    # weights, host-split: K/Q by output-channel halves, V by v-col halves
    wka_d = nc.dram_tensor("wka", [C, 384], BF16, kind="ExternalInput")
    wkb_d = nc.dram_tensor("wkb", [C, 384], BF16, kind="ExternalInput")
    wqa_d = nc.dram_tensor("wqa", [C, 384], BF16, kind="ExternalInput")
    wqb_d = nc.dram_tensor("wqb", [C, 384], BF16, kind="ExternalInput")
    wva_d = nc.dram_tensor("wva", [C, 384], BF16, kind="ExternalInput")
    wvb_d = nc.dram_tensor("wvb", [C, 384], BF16, kind="ExternalInput")
    wp_d = nc.dram_tensor("wp", [C, C], BF16, kind="ExternalInput")
    # K/Q biases packed [128, NCT] f32 (per-partition scalars per ct)
    bkq_d = nc.dram_tensor("bkq", [128, 2 * NCT], F32, kind="ExternalInput")
    bv_d = nc.dram_tensor("bv", [1, C], BF16, kind="ExternalInput")
    bp_d = nc.dram_tensor("bp", [1, C], BF16, kind="ExternalInput")
    mask_d = nc.dram_tensor("mask", [128, NKT * WIN], BF16, kind="ExternalInput")
    out = nc.dram_tensor("out", [TQ, C], BF16, kind="ExternalOutput")

    with TileContext(nc) as tc:
        with (
            tc.tile_pool(name="persist", bufs=1) as pp,
            tc.tile_pool(name="work", bufs=3) as wkp,
            tc.tile_pool(name="et", bufs=2) as et_pool,
            tc.tile_pool(name="psA", bufs=2, space="PSUM") as psA,
            tc.tile_pool(name="psS", bufs=2, space="PSUM") as psS,
            tc.tile_pool(name="psY", bufs=2, space="PSUM") as psY,
        ):
            # ---- persistent SBUF tensors ----
            xt0 = pp.tile([128, NCT, 512], BF16)
            xt1 = pp.tile([128, NCT, 512], BF16)
            xt2 = pp.tile([128, NCT, 256], BF16)
            wka = pp.tile([128, NCT, 384], BF16)
            wkb = pp.tile([128, NCT, 384], BF16)
            wqa = pp.tile([128, NCT, 384], BF16)
            wqb = pp.tile([128, NCT, 384], BF16)
            wva = pp.tile([128, NCT, 384], BF16)
            wvb = pp.tile([128, NCT, 384], BF16)
            wp_sb = pp.tile([128, NCT, C], BF16)
            bkq_sb = pp.tile([128, 2 * NCT], F32)
            bv_sb = pp.tile([1, C], BF16)
            bp_sb = pp.tile([1, C], BF16)
            mask_sb = pp.tile([128, NKT, WIN], BF16)
            ones_sb = pp.tile([1, 512], BF16)

            qt_sb = pp.tile([128, NCT, TQ], BF16)     # Q_T: head dims on partitions
            kt_sb = pp.tile([128, NCT, ROWS], BF16)   # K_T
            v_sb = pp.tile([128, NKT, N_HEAD, 128], BF16)  # [V(64) | ones(64)]
            yn_sb = pp.tile([128, NCT, TQ], BF16)     # normalized Y_T

            # DMAs on two queues: weights/biases on SP, x/mask on Pool
            nc.gpsimd.dma_start(xt0[:], xt0_d.rearrange("(t p) n -> p t n", p=128))
            nc.gpsimd.dma_start(xt1[:], xt1_d.rearrange("(t p) n -> p t n", p=128))
            nc.gpsimd.dma_start(xt2[:], xt2_d.rearrange("(t p) n -> p t n", p=128))
            nc.gpsimd.dma_start(mask_sb[:], mask_d.rearrange("p (k w) -> p k w", w=WIN))
            nc.sync.dma_start(wka[:], wka_d.rearrange("(t p) n -> p t n", p=128))
            nc.sync.dma_start(bkq_sb[:], bkq_d[:])
            nc.sync.dma_start(wqa[:], wqa_d.rearrange("(t p) n -> p t n", p=128))
            nc.sync.dma_start(wva[:], wva_d.rearrange("(t p) n -> p t n", p=128))
            nc.sync.dma_start(bv_sb[:], bv_d[:])
            nc.sync.dma_start(wkb[:], wkb_d.rearrange("(t p) n -> p t n", p=128))
            nc.sync.dma_start(wqb[:], wqb_d.rearrange("(t p) n -> p t n", p=128))
            nc.sync.dma_start(wvb[:], wvb_d.rearrange("(t p) n -> p t n", p=128))
            nc.sync.dma_start(wp_sb[:], wp_d.rearrange("(t p) n -> p t n", p=128))
            nc.sync.dma_start(bp_sb[:], bp_d[:])
            nc.vector.memset(ones_sb[:], 1.0)
            nc.vector.memset(v_sb[:], 1.0)   # contiguous; V halves overwritten

            xchunks = ((xt0, 0, 512), (xt1, 512, 512), (xt2, 1024, 256))

            def w_at(wa, wb, ct):
                w, off = (wa, ct) if ct < 3 else (wb, ct - 3)
                return w, off * 128

            # K_T / Q_T projection for one output-channel tile ct, bias fused
            # into the PSUM->SBUF copy as a per-partition scalar add on DVE.
            def proj_kq(ct):
                for kind in (0, 1):  # 0 = K, 1 = Q
                    wa, wb = (wka, wkb) if kind == 0 else (wqa, wqb)
                    dst = kt_sb if kind == 0 else qt_sb
                    w, woff = w_at(wa, wb, ct)
                    bias = bkq_sb[:, kind * NCT + ct:kind * NCT + ct + 1]
                    for xt, x0, xw in xchunks:
                        if kind == 1:
                            # queries use x cols [HALO, ROWS)
                            lo = max(0, HALO - x0)
                            if lo >= xw:
                                continue
                            src0, ncols, d0 = lo, xw - lo, x0 + lo - HALO
                        else:
                            src0, ncols, d0 = 0, xw, x0
                        ps = psA.tile([128, 512], F32, tag="mm")
                        for c in range(NCT):
                            nc.tensor.matmul(
                                ps[:, :ncols],
                                w[:, c, woff:woff + 128],
                                xt[:, c, src0:src0 + ncols],
                                start=(c == 0),
                                stop=(c == NCT - 1),
                            )
                        nc.vector.tensor_scalar_add(
                            dst[:, ct, d0:d0 + ncols], ps[:, :ncols], bias,
                        )

            # V projection for one 384-wide v-column half (6 heads)
            def proj_v(half):
                wv = wva if half == 0 else wvb
                n0 = half * 384
                for r in range(NKT):
                    xt, x0, _ = xchunks[min(r // 4, 2)]
                    src0 = r * 128 - x0
                    ps = psA.tile([128, 512], F32, tag="mm")
                    nc.tensor.matmul(
                        ps[:, :384], ones_sb[0:1, :128], bv_sb[0:1, n0:n0 + 384],
                        start=True, stop=False,
                    )
                    for c in range(NCT):
                        nc.tensor.matmul(
                            ps[:, :384],
                            xt[:, c, src0:src0 + 128],
                            wv[:, c, :384],
                            start=False,
                            stop=(c == NCT - 1),
                        )
                    h0 = half * 6
                    nc.any.tensor_copy(
                        out=v_sb[:, r, h0:h0 + 6, 0:HD],
                        in_=ps[:, :384].rearrange("p (h d) -> p h d", d=HD),
                    )

            # attention scores + exp for one head -> et tile
            def attn_s(h):
                ct = h // 2
                p0 = (h % 2) * HD
                et = et_pool.tile([128, NKT, WIN], BF16, tag="et")
                for kt2 in range(0, NKT, 2):
                    ps_s = psS.tile([128, 2, 512], F32, tag="ss")
                    for j in range(2):
                        kt = kt2 + j
                        nc.tensor.matmul(
                            ps_s[:, j, :WIN],
                            kt_sb[p0:p0 + HD, ct, kt * 128:(kt + 1) * 128],
                            qt_sb[p0:p0 + HD, ct, QS[kt]:QS[kt] + WIN],
                            start=True, stop=True,
                        )
                    nc.scalar.activation(
                        et[:, kt2:kt2 + 2, :], ps_s[:, :, :WIN], AF.Exp,
                        scale=SCALE,
                    )
                nc.vector.tensor_tensor(et[:], et[:], mask_sb[:], OP.mult)
                return et

            # A*V + softmax denominators + normalize for one head
            def attn_av(h, et):
                ct = h // 2
                p0 = (h % 2) * HD
                for half in range(2):
                    ps_y = psY.tile([128, 512], F32, tag="y")
                    qb0 = half * 4
                    mms = []
                    for kt in range(NKT):
                        for qb in (kt - 2, kt - 1, kt):
                            if qb0 <= qb < qb0 + 4:
                                mms.append((kt, qb))
                    for i, (kt, qb) in enumerate(mms):
                        j0 = qb * 128 - QS[kt]
                        nc.tensor.matmul(
                            ps_y[:, (qb - qb0) * 128:(qb - qb0 + 1) * 128],
                            v_sb[:, kt, h, :],
                            et[:, kt, j0:j0 + 128],
                            start=(i == 0),
                            stop=(i == len(mms) - 1),
                            skip_group_check=True,
                        )
                    # 1/D via exp(-ln(D)): Ln on rows 64-127, Exp(scale=-1)
                    # shift-copies down to rows 0-63.  Same ACT table set.
                    rln = wkp.tile([128, 512], F32, tag="rln")
                    rec = wkp.tile([HD, 512], F32, tag="rec")
                    nc.scalar.activation(rln[HD:128, :], ps_y[HD:128, :], AF.Ln)
                    nc.scalar.activation(rec[:, :], rln[HD:128, :], AF.Exp,
                                         scale=-1.0)
                    nc.vector.tensor_tensor(
                        yn_sb[p0:p0 + HD, ct, half * 512:(half + 1) * 512],
                        ps_y[0:HD, :],
                        rec[:, :],
                        OP.mult,
                    )

            def out_proj(qb):
                o_sb = wkp.tile([128, C], BF16, tag="osb")
                for n0 in (0, 384):
                    ps = psA.tile([128, 512], F32, tag="mm")
                    nc.tensor.matmul(
                        ps[:, :384], ones_sb[0:1, :128], bp_sb[0:1, n0:n0 + 384],
                        start=True, stop=False,
                    )
                    for c in range(NCT):
                        nc.tensor.matmul(
                            ps[:, :384],
                            yn_sb[:, c, qb * 128:(qb + 1) * 128],
                            wp_sb[:, c, n0:n0 + 384],
                            start=False,
                            stop=(c == NCT - 1),
                        )
                    nc.any.tensor_copy(out=o_sb[:, n0:n0 + 384], in_=ps[:, :384])
                nc.sync.dma_start(out[qb * 128:(qb + 1) * 128, :], o_sb[:])

            # ---- issue order: pipeline proj with attention per head-pair ----
            for ct in (0, 1, 2):
                proj_kq(ct)
            proj_v(0)

            ets = {}
            ets[0] = attn_s(0)
            ets[1] = attn_s(1)
            proj_kq(3)
            attn_av(0, ets.pop(0))
            attn_av(1, ets.pop(1))

            ets[2] = attn_s(2)
            ets[3] = attn_s(3)
            proj_kq(4)
            attn_av(2, ets.pop(2))
            attn_av(3, ets.pop(3))

            ets[4] = attn_s(4)
            ets[5] = attn_s(5)
            proj_v(1)
            attn_av(4, ets.pop(4))
            attn_av(5, ets.pop(5))

            ets[6] = attn_s(6)
            ets[7] = attn_s(7)
            proj_kq(5)
            attn_av(6, ets.pop(6))
            attn_av(7, ets.pop(7))

            for h in (8, 9, 10, 11):
                et = attn_s(h)
                attn_av(h, et)

            for qb in range(8):
                out_proj(qb)

    nc.compile()
    return nc


_NC_CACHE = []


def _get_nc() -> bass.Bass:
    if not _NC_CACHE:
        _NC_CACHE.append(_build_nc())
    return _NC_CACHE[0]


def _make_mask(half: int) -> np.ndarray:
    chunk_start = half * TQ
    p = np.arange(128)[:, None, None]
    kt = np.arange(NKT)[None, :, None]
    j = np.arange(WIN)[None, None, :]
    lk = 128 * kt + p
    qi = np.array(QS)[None, :, None] + j
    band = (qi >= lk - WINDOW) & (qi <= lk - 1)
    exists = (chunk_start - HALO + lk) >= 0
    m = (band & exists).astype(ml_dtypes.bfloat16)
    return m.reshape(128, NKT * WIN)


def build_in_maps(x, W_attn, b_attn, W_proj, b_proj):
    x = np.asarray(x, dtype=np.float32)
    W_attn = np.asarray(W_attn, dtype=np.float32)
    b_attn = np.asarray(b_attn, dtype=np.float32)
    W_proj = np.asarray(W_proj, dtype=np.float32)
    b_proj = np.asarray(b_proj, dtype=np.float32)

    bf = ml_dtypes.bfloat16
    wq_h = W_attn[:, 0:C].astype(bf)
    wk_h = W_attn[:, C:2 * C].astype(bf)
    wv_h = W_attn[:, 2 * C:3 * C].astype(bf)
    wp_h = np.ascontiguousarray(W_proj).astype(bf)
    bq_h = b_attn[0:C].astype(np.float32)
    bk_h = b_attn[C:2 * C].astype(np.float32)
    # [128, 2*NCT]: [:, 0:6] = K bias per ct, [:, 6:12] = Q bias per ct
    bkq_h = np.concatenate(
        [bk_h.reshape(NCT, 128).T, bq_h.reshape(NCT, 128).T], axis=1
    ).astype(np.float32)
    bv_h = b_attn[2 * C:3 * C].reshape(1, C).astype(bf)
    bp_h = b_proj.reshape(1, C).astype(bf)
    masks = [_make_mask(0), _make_mask(1)]

    halves = {
        "wka": np.ascontiguousarray(wk_h[:, 0:384]),
        "wkb": np.ascontiguousarray(wk_h[:, 384:768]),
        "wqa": np.ascontiguousarray(wq_h[:, 0:384]),
        "wqb": np.ascontiguousarray(wq_h[:, 384:768]),
        "wva": np.ascontiguousarray(wv_h[:, 0:384]),
        "wvb": np.ascontiguousarray(wv_h[:, 384:768]),
    }

    in_maps = []
    for core in range(8):
        b, half = divmod(core, 2)
        start = half * TQ - HALO
        if start < 0:
            x_win = np.concatenate(
                [np.zeros((HALO, C), np.float32), x[b, 0:TQ]], axis=0)
        else:
            x_win = x[b, start:start + ROWS]
        x_t = np.ascontiguousarray(x_win.T).astype(bf)
        in_maps.append({
            "xt0": np.ascontiguousarray(x_t[:, 0:512]),
            "xt1": np.ascontiguousarray(x_t[:, 512:1024]),
            "xt2": np.ascontiguousarray(x_t[:, 1024:1280]),
            **halves,
            "wp": wp_h, "bkq": bkq_h, "bv": bv_h, "bp": bp_h,
            "mask": masks[half],
        })
    return in_maps


def kernel(x, W_attn, b_attn, W_proj, b_proj):
    in_maps = build_in_maps(x, W_attn, b_attn, W_proj, b_proj)
    nc = _get_nc()
    res = run_bass_kernel_spmd(nc, in_maps, list(range(8)))
    y = np.empty((B, T, C), dtype=np.float32)
    for core in range(8):
        b, half = divmod(core, 2)
        y[b, half * TQ:(half + 1) * TQ, :] = res.results[core]["out"].astype(
            np.float32)
    return y
